# revision 8
# baseline (speedup 1.0000x reference)
"""Associative-embedding loss (push/pull) on 8 TRN2 NeuronCores.

Strategy (pure data parallel, hardcoded):
  - B=32 images, 8 cores -> 4 images per core.
  - Per image only 510 of the 278528 tag rows are needed, so the kernel
    never streams the tags tensor. The rows are fetched with FOUR
    multi-descriptor `dma_gather` instructions (one per image, 512
    descriptors each, single_packet=False) - measured ~4.3 us each with
    ~40 ns gaps, vs ~1.41 us per 128-descriptor indirect DMA (16
    needed): the chain is ~17 us instead of ~22.5 us, and the ~6 us
    one-time Q7 IRAM load for the gather ucode overlaps the index DMA
    and the other engines' preambles.
  - dma_gather requires 256B-aligned elements and int16 indices, so
    each descriptor fetches the 256B chunk (16 tag rows) containing the
    keypoint's row (chunk index = row//16 < 17408 fits int16 per
    image); the wanted 16B row is extracted with a host-built one-hot
    mask multiply + strided reduce on DVE, pipelined under the later
    gathers.
  - Per-image compute pipelines under the gather chain. Visibility is
    folded into the per-image membership matrices host-side, so the
    slot matmuls consume extracted rows directly; the per-image chain:
      msum (4 matmuls) -> meanT / ACT-square / -2*mean (parallel) ->
      q (1 matmul) -> qrow -> diff (3 matmuls) -> exp / mask ->
      fused mask-multiply + row-reduce.
  - Final push/pull scalars via two ones-matmuls + small DVE ops.
  - Everything that depends only on `keypoints` (visibility, counts,
    pair masks, extraction masks, scales) is precomputed on the host.

Inputs: tags [32, 278528, 4] f32, keypoints [32, 30, 17, 2] int.
Output: [32, 2] f32 (push, pull) per image.
"""

import numpy as np

import concourse.bacc as bacc
import concourse.bass as bass
import concourse.mybir as mybir
import concourse.tile as tile
from concourse.bass_utils import run_bass_kernel_spmd

B, N, D = 32, 278528, 4
NPERS, NKP = 30, 17
NFLAT = NPERS * NKP          # 510 keypoints per image
KPJ = 4                      # keypoint slot-columns (4 x 128 = 512 slots)
NCORES = 8
IMGS = B // NCORES           # 4 images per core
W = IMGS * NPERS             # 120: merged free width
CHW = 16                     # tag rows per 256B gather chunk
ELEM = CHW * D               # 64 floats per gather chunk
NCH = N // CHW               # 17408 chunks per image (fits int16)
NI = 128 * KPJ               # 512 descriptors per image
EPS = 1e-6
X = mybir.AxisListType


def _build_nc():
    nc = bacc.Bacc("TRN2", target_bir_lowering=False, debug=False)
    f32 = mybir.dt.float32
    tags = nc.dram_tensor("tags", [IMGS * N, D], f32, kind="ExternalInput").ap()
    idx16 = nc.dram_tensor(
        "idx16", [128, IMGS * (NI // 16)], mybir.dt.int16, kind="ExternalInput"
    ).ap()
    emask = nc.dram_tensor("emask", [128, IMGS * KPJ * ELEM], f32, kind="ExternalInput").ap()
    # member3 with vis folded: cols b*120 + j*30 + person; then w2 cols
    # 480 + b*16 + j*4 + d   (slot s of image = j*128 + p)
    mvw = nc.dram_tensor("mvw", [128, IMGS * W + IMGS * KPJ * D], f32, kind="ExternalInput").ap()
    # rows 0:30 cols 0:120 hmask; rows 0:4: cols 120:240 inv4 (1/cnt),
    # cols 240:360 n2inv4 (-2/cnt); row 0: cols 360:480 inv2 (1/cnt^2),
    # cols 480:488 scales (push x4, pull x4)
    cst2 = nc.dram_tensor("cst2", [30, 488], f32, kind="ExternalInput").ap()
    out = nc.dram_tensor("out", [1, IMGS * 2], f32, kind="ExternalOutput").ap()

    tags_ch = tags.rearrange("(a c) d -> a (c d)", c=CHW)   # [IMGS*NCH, 64]

    with tile.TileContext(nc) as tc:
        with (
            tc.tile_pool(name="const", bufs=1) as cpool,
            tc.tile_pool(name="work", bufs=4) as wpool,
            tc.tile_pool(name="psum", bufs=1, space="PSUM") as ppool,
        ):
            # critical path: index load on sync (alone), then the 4 gathers
            idx_t = cpool.tile([128, IMGS * (NI // 16)], mybir.dt.int16)
            nc.sync.dma_start(idx_t[:], idx16)
            ch_t = cpool.tile([128, IMGS * KPJ * ELEM], f32)
            for b in range(IMGS):
                nc.gpsimd.dma_gather(
                    ch_t[:, b * KPJ * ELEM:(b + 1) * KPJ * ELEM].rearrange(
                        "p (k f) -> p k f", f=ELEM
                    ),
                    tags_ch[b * NCH:(b + 1) * NCH, :],
                    idx_t[:, b * (NI // 16):(b + 1) * (NI // 16)],
                    NI, NI, ELEM,
                    single_packet=False,
                )

            emask_t = cpool.tile([128, IMGS * KPJ * ELEM], f32)
            nc.scalar.dma_start(emask_t[:], emask)
            mvw_t = cpool.tile([128, IMGS * W + IMGS * KPJ * D], f32)
            nc.scalar.dma_start(mvw_t[:], mvw)
            cst2_t = cpool.tile([30, 488], f32)
            nc.scalar.dma_start(cst2_t[:], cst2)
            hmask = cst2_t[0:30, 0:W]
            inv4 = cst2_t[0:D, W:2 * W]
            n2inv4 = cst2_t[0:D, 2 * W:3 * W]
            inv2 = cst2_t[0:1, 3 * W:4 * W]
            scales = cst2_t[0:1, 4 * W:4 * W + 2 * IMGS]

            ones4_t = cpool.tile([D, 1], f32)
            nc.vector.memset(ones4_t[:], 1.0)
            ones30_t = cpool.tile([NPERS, 1], f32)
            nc.vector.memset(ones30_t[:], 1.0)
            ones128_t = cpool.tile([128, 1], f32)
            nc.vector.memset(ones128_t[:], 1.0)
            ones1_t = cpool.tile([1, NPERS], f32)
            nc.vector.memset(ones1_t[:], 1.0)

            cols_t = cpool.tile([128, IMGS], f32)
            meanT_t = cpool.tile([D, W], f32)
            sqm_t = cpool.tile([D, W], f32)
            n2m_t = cpool.tile([D, W], f32)
            qrow_t = cpool.tile([1, W], f32)
            e_t = cpool.tile([NPERS, W], f32)
            m2_t = cpool.tile([NPERS, W], f32)
            c_t = cpool.tile([NPERS, W], f32)
            prow_t = cpool.tile([NPERS, IMGS], f32)
            dall_p = ppool.tile([NPERS, W], f32, space="PSUM")

            # per-image pipeline (images 0..2 hide under the gather chain)
            for b in range(IMGS):
                c30 = slice(b * NPERS, (b + 1) * NPERS)
                csl = slice(b * KPJ * ELEM, (b + 1) * KPJ * ELEM)
                # extract the wanted 16B row from each 256B chunk
                mm_t = wpool.tile([128, KPJ * ELEM], f32, tag="mm", bufs=2)
                nc.vector.tensor_mul(mm_t[:], ch_t[:, csl], emask_t[:, csl])
                g_t = wpool.tile([128, KPJ * D], f32, tag="g")
                nc.vector.reduce_sum(
                    g_t[:].rearrange("p (j d) -> p j d", j=KPJ),
                    mm_t[:].rearrange("p (j r d) -> p j d r", j=KPJ, r=CHW),
                    axis=X.X,
                )
                msum_p = ppool.tile([D, NPERS], f32, space="PSUM", tag="msum", bufs=2)
                for j in range(KPJ):
                    nc.tensor.matmul(
                        out=msum_p[:],
                        lhsT=g_t[:, j * D:(j + 1) * D],
                        rhs=mvw_t[:, b * W + j * NPERS:b * W + (j + 1) * NPERS],
                        start=(j == 0),
                        stop=(j == KPJ - 1),
                    )
                # pull term 1: sum over slots of vis*invcnt*|row|^2
                w2sl = slice(
                    IMGS * W + b * KPJ * D, IMGS * W + (b + 1) * KPJ * D
                )
                u_t = wpool.tile([128, KPJ * D], f32, tag="u", bufs=2)
                nc.vector.tensor_mul(u_t[:], g_t[:], mvw_t[:, w2sl])
                u2_t = wpool.tile([128, KPJ * D], f32, tag="u2", bufs=2)
                nc.vector.tensor_mul(u2_t[:], u_t[:], g_t[:])
                nc.vector.reduce_sum(cols_t[:, b:b + 1], u2_t[:], axis=X.X)
                # means and q pieces (ACT square in parallel with DVE muls)
                nc.vector.tensor_mul(meanT_t[:, c30], msum_p[:], inv4[:, c30])
                nc.scalar.square(sqm_t[:, c30], msum_p[:])
                nc.vector.tensor_mul(n2m_t[:, c30], msum_p[:], n2inv4[:, c30])
                q_p = ppool.tile([1, NPERS], f32, space="PSUM", tag="q", bufs=2)
                nc.tensor.matmul(
                    out=q_p[:], lhsT=ones4_t[:], rhs=sqm_t[:, c30],
                    start=True, stop=True,
                )
                nc.vector.tensor_mul(qrow_t[:, c30], q_p[:], inv2[:, c30])
                # diff[i,j] = q_i + q_j - 2<mi,mj>
                nc.tensor.matmul(
                    out=dall_p[:, c30], lhsT=n2m_t[:, c30], rhs=meanT_t[:, c30],
                    start=True, stop=False,
                )
                nc.tensor.matmul(
                    out=dall_p[:, c30], lhsT=qrow_t[:, c30], rhs=ones1_t[:],
                    start=False, stop=False,
                )
                nc.tensor.matmul(
                    out=dall_p[:, c30], lhsT=ones1_t[:], rhs=qrow_t[:, c30],
                    start=False, stop=True,
                )
                # push piece: exp(-diff) * (diff != 0) * hmask, row-reduced
                nc.scalar.activation(
                    e_t[:, c30], dall_p[:, c30],
                    mybir.ActivationFunctionType.Exp, bias=0.0, scale=-1.0,
                )
                nc.vector.scalar_tensor_tensor(
                    m2_t[:, c30], dall_p[:, c30], 0.0, hmask[:, c30],
                    op0=mybir.AluOpType.not_equal, op1=mybir.AluOpType.mult,
                )
                nc.vector.scalar_tensor_tensor(
                    c_t[:, c30], e_t[:, c30], 1.0, m2_t[:, c30],
                    op0=mybir.AluOpType.mult, op1=mybir.AluOpType.mult,
                    accum_out=prow_t[:, b:b + 1],
                )

            # epilogue: push = scale * sum_person prow, pull = scale * (t1 - term2)
            pt_p = ppool.tile([1, IMGS], f32, space="PSUM")
            nc.tensor.matmul(
                out=pt_p[:], lhsT=ones30_t[:], rhs=prow_t[:], start=True, stop=True
            )
            t1_p = ppool.tile([1, IMGS], f32, space="PSUM")
            nc.tensor.matmul(
                out=t1_p[:], lhsT=ones128_t[:], rhs=cols_t[:], start=True, stop=True
            )
            term2_t = cpool.tile([1, IMGS], f32)
            nc.vector.reduce_sum(
                term2_t[:], qrow_t[:].rearrange("o (i p) -> o i p", p=NPERS),
                axis=X.X,
            )
            pull4_t = cpool.tile([1, IMGS], f32)
            nc.vector.tensor_sub(pull4_t[:], t1_p[:], term2_t[:])
            res_t = cpool.tile([1, IMGS * 2], f32)
            r3 = res_t[:].rearrange("o (i t) -> o i t", t=2)
            nc.vector.tensor_mul(
                r3[:, :, 0:1],
                pt_p[:].rearrange("o (i u) -> o i u", u=1),
                scales[:, 0:IMGS].rearrange("o (i u) -> o i u", u=1),
            )
            nc.vector.tensor_mul(
                r3[:, :, 1:2],
                pull4_t[:].rearrange("o (i u) -> o i u", u=1),
                scales[:, IMGS:2 * IMGS].rearrange("o (i u) -> o i u", u=1),
            )
            nc.sync.dma_start(out, res_t[:])

    nc.compile()
    return nc


_NC_CACHE = None


def _get_nc():
    global _NC_CACHE
    if _NC_CACHE is None:
        _NC_CACHE = _build_nc()
    return _NC_CACHE


def _host_prep(tags: np.ndarray, keypoints: np.ndarray):
    """Build the per-core input maps. tags [B,N,D] f32, keypoints [B,30,17,2].

    Keypoint m (0..509) of an image maps to descriptor m of that image's
    dma_gather: partition m%128, slot-column j=m//128. person(m)=m//17.
    """
    kp_idx = keypoints[..., 0].reshape(B, NFLAT).astype(np.int64)
    kp_vis = (keypoints[..., 1] > 0).reshape(B, NFLAT)
    upper = np.triu(np.ones((NPERS, NPERS), dtype=bool), 1)
    m_all = np.arange(NI)
    m_part = m_all % 128
    m_col = m_all // 128
    m_person = np.minimum(m_all // NKP, NPERS - 1)
    m_valid = m_all < NFLAT

    in_maps = []
    for c in range(NCORES):
        tags_flat = np.ascontiguousarray(
            tags[c * IMGS:(c + 1) * IMGS].reshape(IMGS * N, D), dtype=np.float32
        )
        idx16 = np.zeros((128, IMGS * (NI // 16)), dtype=np.int16)
        emask = np.zeros((128, IMGS * KPJ * ELEM), dtype=np.float32)
        mvw = np.zeros((128, IMGS * W + IMGS * KPJ * D), dtype=np.float32)
        cst2 = np.zeros((30, 488), dtype=np.float32)
        for lb in range(IMGS):
            gb = c * IMGS + lb
            fidx = kp_idx[gb]            # [510] tag-row index per keypoint
            fvis = kp_vis[gb]            # [510]
            rows = np.zeros(NI, dtype=np.int64)
            rows[:NFLAT] = fidx
            vis = np.zeros(NI, dtype=np.float32)
            vis[:NFLAT] = fvis.astype(np.float32)
            # gather indices: descriptor m -> chunk rows[m]//16, wrapped
            # [m%16, m//16] within this image's 32-column block
            blk = np.zeros((16, NI // 16), dtype=np.int16)
            blk[m_all % 16, m_all // 16] = (rows // CHW).astype(np.int16)
            idx16[:, lb * (NI // 16):(lb + 1) * (NI // 16)] = np.tile(blk, (8, 1))
            # extraction one-hot: [p, j*ELEM + (row%16)*4 + d] = valid
            r_in = rows % CHW
            for d in range(D):
                emask[
                    m_part, lb * KPJ * ELEM + m_col * ELEM + r_in * D + d
                ] = m_valid
            vis_pk = fvis.reshape(NPERS, NKP)
            cnt = vis_pk.sum(axis=1).astype(np.float32)
            valid = cnt > 0
            safe_cnt = np.maximum(cnt, 1.0)
            invcv = valid / safe_cnt
            mv = vis * m_valid
            mvw[m_part, lb * W + m_col * NPERS + m_person] = mv
            w2v = mv * invcv[m_person]
            for d in range(D):
                mvw[m_part, IMGS * W + lb * KPJ * D + m_col * D + d] = w2v
            cst2[0:D, W + lb * NPERS:W + (lb + 1) * NPERS] = (1.0 / safe_cnt)[None, :]
            cst2[0:D, 2 * W + lb * NPERS:2 * W + (lb + 1) * NPERS] = (
                -2.0 / safe_cnt
            )[None, :]
            cst2[0, 3 * W + lb * NPERS:3 * W + (lb + 1) * NPERS] = 1.0 / (
                safe_cnt * safe_cnt
            )
            cst2[0:NPERS, lb * NPERS:(lb + 1) * NPERS] = (
                upper & valid[:, None] & valid[None, :]
            ).astype(np.float32)
            n = valid.sum().astype(np.float32)
            cst2[0, 4 * W + lb] = 1.0 / ((n - 1.0) * n + EPS)
            cst2[0, 4 * W + IMGS + lb] = 1.0 / (n + EPS)
        in_maps.append(
            {
                "tags": tags_flat,
                "idx16": idx16,
                "emask": emask,
                "mvw": mvw,
                "cst2": cst2,
            }
        )
    return in_maps


def kernel(tags: np.ndarray, keypoints: np.ndarray) -> np.ndarray:
    tags = np.asarray(tags, dtype=np.float32)
    keypoints = np.asarray(keypoints)
    nc = _get_nc()
    in_maps = _host_prep(tags, keypoints)
    res = run_bass_kernel_spmd(nc, in_maps, core_ids=list(range(NCORES)))
    outs = [np.asarray(r["out"]).reshape(IMGS, 2) for r in res.results]
    return np.concatenate(outs, axis=0)
